# revision 15
# baseline (speedup 1.0000x reference)
"""CRF negative log-likelihood on 8 Trainium2 NeuronCores.

Strategy (pure data parallel, batch sharded 1024 -> 8 x 128):

  The forward-algorithm recursion is run in the exp domain as
      P_{t} = Q_t (.) (M P_{t-1}),   Q = exp(feat - DELTA),  M = exp(T)
  i.e. one tiny PE matmul + one DVE multiply per step.  To double
  instruction-level parallelism and halve the serial chain length, the
  sequence is processed simultaneously from BOTH ends:
      forward  :  P_t  = Q_t (.) (M P_{t-1})          t = 1..255
      backward :  bv_t = M^T (Q_{t+1} (.) bv_{t+1})   t = 510..255
      Z_b = sum_j P_255[j,b] * bv_255[j,b]
  The fwd state lives on SBUF partitions 0..47 and the bwd state on
  64..111, so each tick needs ONE [128,64] DVE multiply per column
  stream (2 streams of 64 batch columns hide the PE<->DVE latency).
  Matmuls keep contraction k=48 on the partition ranges 0-47 / 64-111
  via PE tile_position quadrants, so the junk lanes between them are
  never consumed.

  Q is produced in natural layout by the scalar engine (exp with bias)
  into a 64-padded buffer and moved to [tag, batch] layout by ONE big
  multi-tile xbar DMA-transpose per 64-step chunk-pair (bwd steps are
  written in reversed order so transposed blocks come out tick-indexed).
  Exact renormalization once at mid-chain keeps bf16/fp32 in range; the
  log-corrections are added to logz at the end.

  The gold-path score (emit gather + transition lookups) only needs
  numpy gathers over host-resident inputs (~25 ms) and is computed on
  the host, keeping the device kernel purely the log-partition.
"""

import numpy as np
import ml_dtypes

B, S, T = 1024, 512, 48
NCORES = 8
BC = B // NCORES          # 128 batch rows per core
DELTA = 5.0               # deterministic per-step log shift
NT = 255                  # chain ticks (fwd 255 steps + bwd 256 steps)
CHUNK = 64                # steps per feat chunk
NPAIR = 4                 # chunk pairs (fwd c, bwd 7-c)
PAD = 128                 # qpad block width: fwd 48 | pad | bwd 48 | pad

BF16 = ml_dtypes.bfloat16

_NC = None


def _build_nc():
    import concourse.mybir as mybir
    import concourse.tile as tile
    from concourse import bacc

    f32 = mybir.dt.float32
    bf16 = mybir.dt.bfloat16
    Act = mybir.ActivationFunctionType
    Alu = mybir.AluOpType

    nc = bacc.Bacc()

    feats_d = nc.declare_dram_parameter("feats", [BC, S * T], f32, isOutput=False)
    wf_d = nc.declare_dram_parameter("wf", [T, 64], bf16, isOutput=False)
    wb_d = nc.declare_dram_parameter("wb", [T, 64], bf16, isOutput=False)
    initsc_d = nc.declare_dram_parameter("initsc", [128, 1], f32, isOutput=False)
    ones48_d = nc.declare_dram_parameter("ones48", [T, 1], bf16, isOutput=False)
    ones64_d = nc.declare_dram_parameter("ones64", [1, 64], bf16, isOutput=False)
    negdelta_d = nc.declare_dram_parameter("negdelta", [128, 1], f32, isOutput=False)
    zero1_d = nc.declare_dram_parameter("zero1", [1, 1], f32, isOutput=False)
    logz_d = nc.declare_dram_parameter("logz", [1, BC], f32, isOutput=True)

    with tile.TileContext(nc) as tc:
        with (
            tc.tile_pool(name="const", bufs=1) as cpool,
            tc.tile_pool(name="qtp", bufs=1) as qtp,
            tc.tile_pool(name="feat", bufs=2) as fpool,
            tc.tile_pool(name="qpad", bufs=2) as qpool,
            tc.tile_pool(name="state", bufs=3) as spool,
            tc.tile_pool(name="small", bufs=2) as mpool,
            tc.tile_pool(name="psum", bufs=2, space="PSUM") as psum,
            tc.tile_pool(name="psmall", bufs=2, space="PSUM") as pss,
        ):
            # ---- constants ----
            wf_sb = cpool.tile([T, 64], bf16, name="wf_sb")
            nc.sync.dma_start(wf_sb[:, :], wf_d[:, :])
            wb_sb = cpool.tile([112, 64], bf16, name="wb_sb")
            nc.sync.dma_start(wb_sb[64:112, :], wb_d[:, :])
            initsc_sb = cpool.tile([128, 1], f32, name="initsc_sb")
            nc.sync.dma_start(initsc_sb[:, :], initsc_d[:, :])
            ones48_sb = cpool.tile([T, 1], bf16, name="ones48_sb")
            nc.sync.dma_start(ones48_sb[:, :], ones48_d[:, :])
            ones48b_sb = cpool.tile([112, 1], bf16, name="ones48b_sb")
            nc.sync.dma_start(ones48b_sb[64:112, :], ones48_d[:, :])
            ones64_sb = cpool.tile([1, 64], bf16, name="ones64_sb")
            nc.sync.dma_start(ones64_sb[:, :], ones64_d[:, :])
            negdelta_sb = cpool.tile([128, 1], f32, name="negdelta_sb")
            nc.sync.dma_start(negdelta_sb[:, :], negdelta_d[:, :])
            zero1_sb = cpool.tile([1, 1], f32, name="zero1_sb")
            nc.sync.dma_start(zero1_sb[:, :], zero1_d[:, :])

            # ---- transposed Q storage: block j cols [128j:128j+128):
            #      rows 0-47 = Q^T(fwd step j), rows 64-111 = Q^T(bwd 511-j)
            qt = qtp.tile([128, 256 * PAD], bf16, name="qt")
            qt3 = qt.rearrange("p (b c) -> p b c", c=PAD)

            # ---- per chunk-pair prep: DMA feats, exp, pad-memset, transpose
            for c in range(NPAIR):
                ff = fpool.tile([BC, CHUNK * T], f32, tag="ff", name="ff")
                nc.sync.dma_start(
                    ff[:, :],
                    feats_d[:, c * CHUNK * T:(c + 1) * CHUNK * T],
                )
                fb = fpool.tile([BC, CHUNK * T], f32, tag="fb", name="fb")
                nc.sync.dma_start(
                    fb[:, :],
                    feats_d[:, (7 - c) * CHUNK * T:(8 - c) * CHUNK * T],
                )
                ff3 = ff.rearrange("p (s j) -> p s j", j=T)
                fb3 = fb.rearrange("p (s j) -> p s j", j=T)

                qp = qpool.tile([BC, CHUNK * PAD], bf16, tag="qp", name="qp")
                qp3 = qp.rearrange("p (s k) -> p s k", k=PAD)
                nc.gpsimd.memset(qp3[:, :, T:64], 0.0)
                nc.gpsimd.memset(qp3[:, :, 64 + T:PAD], 0.0)
                nc.scalar.activation(
                    qp3[:, :, 0:T], ff3[:, :, :], Act.Exp,
                    bias=negdelta_sb[:, :],
                )
                # bwd steps reversed so block j holds bwd step 511-j
                nc.scalar.activation(
                    qp3[:, :, 64:64 + T], fb3[:, ::-1, :], Act.Exp,
                    bias=negdelta_sb[:, :],
                )
                nc.scalar.dma_start(
                    qt3[:, c * CHUNK:(c + 1) * CHUNK, :], qp[:, :],
                    transpose=True,
                )

            # ---- init states: P_0 = Q_0 * expstart ; X_511 = Q_511 * expstop
            states = []
            for s2 in range(2):
                cols = slice(64 * s2, 64 * s2 + 64)
                st = spool.tile([128, 64], bf16, tag=f"st{s2}", name=f"st{s2}")
                nc.vector.tensor_scalar(
                    st[:, :], qt3[:, 0, cols], initsc_sb[:, :], None, Alu.mult
                )
                states.append(st)
            cacc = [None, None]

            # ---- the chain ----
            for k in range(NT):
                for s2 in range(2):
                    cols = slice(64 * s2, 64 * s2 + 64)
                    st = states[s2]
                    mm = psum.tile([128, 64], f32, tag=f"mm{s2}", name=f"mm{s2}")
                    nc.tensor.matmul(
                        mm[0:64, :], wf_sb[:, :], st[0:T, :],
                        start=True, stop=True, tile_position=(0, 0),
                    )
                    nc.tensor.matmul(
                        mm[64:128, :], wb_sb[64:112, :], st[64:64 + T, :],
                        start=True, stop=True, tile_position=(64, 64),
                    )
                    nst = spool.tile([128, 64], bf16, tag=f"st{s2}",
                                     name=f"nst{s2}")
                    nc.vector.tensor_tensor(
                        nst[:, :], mm[:, :], qt3[:, k + 1, cols], Alu.mult
                    )
                    states[s2] = nst

                    if k == 127:
                        # exact renorm: scale both halves by 1/colsum,
                        # bank the log-correction
                        st = states[s2]
                        msf = pss.tile([1, 64], f32, tag="ps_a", name="msf")
                        nc.tensor.matmul(
                            msf[:, :], ones48_sb[:, :], st[0:T, :],
                            start=True, stop=True, tile_position=(0, 0),
                        )
                        msb = pss.tile([1, 64], f32, tag="ps_b", name="msb")
                        nc.tensor.matmul(
                            msb[:, :], ones48b_sb[64:112, :], st[64:64 + T, :],
                            start=True, stop=True, tile_position=(64, 0),
                        )
                        lnf = mpool.tile([1, 64], f32, tag=f"lnf{s2}", name="lnf")
                        nc.scalar.activation(lnf[:, :], msf[:, :], Act.Ln, bias=zero1_sb[:, :])
                        lnb = mpool.tile([1, 64], f32, tag=f"lnb{s2}", name="lnb")
                        nc.scalar.activation(lnb[:, :], msb[:, :], Act.Ln, bias=zero1_sb[:, :])
                        cs = mpool.tile([1, 64], f32, tag=f"cs{s2}", name="cs")
                        nc.vector.tensor_add(cs[:, :], lnf[:, :], lnb[:, :])
                        cacc[s2] = cs
                        rf = mpool.tile([1, 64], f32, tag=f"rf{s2}", name="rf")
                        nc.vector.reciprocal(rf[:, :], msf[:, :])
                        rb = mpool.tile([1, 64], f32, tag=f"rb{s2}", name="rb")
                        nc.vector.reciprocal(rb[:, :], msb[:, :])
                        rfb = mpool.tile([1, 64], bf16, tag=f"rfb{s2}", name="rfb")
                        nc.vector.tensor_copy(rfb[:, :], rf[:, :])
                        rbb = mpool.tile([1, 64], bf16, tag=f"rbb{s2}", name="rbb")
                        nc.vector.tensor_copy(rbb[:, :], rb[:, :])
                        sc = psum.tile([128, 64], f32, tag=f"mm{s2}", name="sc")
                        nc.tensor.matmul(
                            sc[0:64, :], ones64_sb[:, :], rfb[:, :],
                            start=True, stop=True, tile_position=(0, 0),
                        )
                        nc.tensor.matmul(
                            sc[64:128, :], ones64_sb[:, :], rbb[:, :],
                            start=True, stop=True, tile_position=(0, 64),
                        )
                        nst2 = spool.tile([128, 64], bf16, tag=f"st{s2}",
                                          name=f"rst{s2}")
                        nc.vector.tensor_tensor(
                            nst2[:, :], sc[:, :], st[:, :], Alu.mult
                        )
                        states[s2] = nst2

            # ---- finalize: bv_255 = M^T X_256 ; Z = sum P_255 (.) bv_255
            logz_sb = cpool.tile([1, BC], f32, name="logz_sb")
            for s2 in range(2):
                cols = slice(64 * s2, 64 * s2 + 64)
                st = states[s2]
                bv = psum.tile([128, 64], f32, tag=f"mm{s2}", name="bv")
                nc.tensor.matmul(
                    bv[0:64, :], wb_sb[64:112, :], st[64:64 + T, :],
                    start=True, stop=True, tile_position=(64, 0),
                )
                zel = mpool.tile([T, 64], bf16, tag=f"zel{s2}", name="zel")
                nc.vector.tensor_tensor(
                    zel[:, :], bv[0:T, :], st[0:T, :], Alu.mult
                )
                zs = pss.tile([1, 64], f32, tag="ps_a", name="zs")
                nc.tensor.matmul(
                    zs[:, :], ones48_sb[:, :], zel[:, :],
                    start=True, stop=True, tile_position=(0, 0),
                )
                lnz = mpool.tile([1, 64], f32, tag=f"lnz{s2}", name="lnz")
                nc.scalar.activation(lnz[:, :], zs[:, :], Act.Ln, bias=zero1_sb[:, :])
                nc.vector.tensor_add(
                    logz_sb[:, cols], lnz[:, :], cacc[s2][:, :]
                )
            nc.sync.dma_start(logz_d[:, :], logz_sb[:, :])

    if not nc.is_finalized():
        nc.finalize()
    return nc


def _get_nc():
    global _NC
    if _NC is None:
        _NC = _build_nc()
    return _NC


def _in_maps(feats):
    Tm = _PARAMS["Tm"]
    st = _PARAMS["st"]
    sp = _PARAMS["sp"]
    M = np.exp(Tm)                          # M[next, cur]
    wf = np.zeros((T, 64), dtype=np.float32)
    wf[:, 0:T] = M.T                        # lhsT for out = M @ x
    wb = np.zeros((T, 64), dtype=np.float32)
    wb[:, 0:T] = M                          # lhsT for out = M.T @ x
    initsc = np.zeros((128, 1), dtype=np.float32)
    initsc[0:T, 0] = np.exp(st)
    initsc[64:64 + T, 0] = np.exp(sp)
    ones48 = np.ones((T, 1), dtype=BF16)
    ones64 = np.ones((1, 64), dtype=BF16)

    maps = []
    for i in range(NCORES):
        sl = slice(i * BC, (i + 1) * BC)
        maps.append(dict(
            feats=feats[sl].reshape(BC, S * T),
            wf=wf.astype(BF16),
            wb=wb.astype(BF16),
            initsc=initsc,
            ones48=ones48,
            ones64=ones64,
            negdelta=np.full((128, 1), -DELTA, dtype=np.float32),
            zero1=np.zeros((1, 1), dtype=np.float32),
        ))
    return maps


_PARAMS = {}


def kernel(feats, tags, mask, transitions, start_transitions, stop_transitions,
           _trace_tmpdir=None):
    from concourse.bass_utils import run_bass_kernel_spmd

    feats = np.ascontiguousarray(np.asarray(feats, dtype=np.float32))
    tags = np.asarray(tags).astype(np.int64)
    Tm = np.asarray(transitions, dtype=np.float32)
    st = np.asarray(start_transitions, dtype=np.float32)
    sp = np.asarray(stop_transitions, dtype=np.float32)
    _PARAMS.update(Tm=Tm, st=st, sp=sp)

    # host: full gold-path score (emit gather + transition lookups)
    emit = np.take_along_axis(
        feats.reshape(B, S * T),
        np.arange(S, dtype=np.int64)[None, :] * T + tags, axis=1,
    ).sum(axis=1, dtype=np.float64)
    gold = (
        emit
        + Tm[tags[:, 1:], tags[:, :-1]].sum(axis=1, dtype=np.float64)
        + st[tags[:, 0]].astype(np.float64)
        + sp[tags[:, -1]].astype(np.float64)
    )

    nc = _get_nc()
    maps = _in_maps(feats)
    if _trace_tmpdir is not None:
        return run_bass_kernel_spmd(nc, maps, list(range(NCORES)),
                                    trace=True, tmpdir=_trace_tmpdir)
    res = run_bass_kernel_spmd(nc, maps, list(range(NCORES))).results

    logz = np.concatenate([r["logz"][0].astype(np.float64) for r in res])
    logz = logz + S * DELTA
    loss = np.mean(logz - gold)
    return np.float32(loss)


# revision 16
# speedup vs baseline: 1.0183x; 1.0183x over previous
"""CRF negative log-likelihood on 8 Trainium2 NeuronCores.

Strategy (pure data parallel, batch sharded 1024 -> 8 x 128):

  The forward-algorithm recursion is run in the exp domain as
      P_{t} = Q_t (.) (M P_{t-1}),   Q = exp(feat - DELTA),  M = exp(T)
  i.e. one tiny PE matmul + one DVE multiply per step.  To double
  instruction-level parallelism and halve the serial chain length, the
  sequence is processed simultaneously from BOTH ends:
      forward  :  P_t  = Q_t (.) (M P_{t-1})          t = 1..255
      backward :  bv_t = M^T (Q_{t+1} (.) bv_{t+1})   t = 510..255
      Z_b = sum_j P_255[j,b] * bv_255[j,b]
  The fwd state lives on SBUF partitions 0..47 and the bwd state on
  64..111, so each tick needs ONE [128,64] DVE multiply per column
  stream (2 streams of 64 batch columns hide the PE<->DVE latency).
  Matmuls keep contraction k=48 on the partition ranges 0-47 / 64-111
  via PE tile_position quadrants, so the junk lanes between them are
  never consumed.

  Q is produced in natural layout by the scalar engine (exp with bias)
  into a 64-padded buffer and moved to [tag, batch] layout by ONE big
  multi-tile xbar DMA-transpose per 64-step chunk-pair (bwd steps are
  written in reversed order so transposed blocks come out tick-indexed).
  Exact renormalization once at mid-chain keeps bf16/fp32 in range; the
  log-corrections are added to logz at the end.

  The gold-path score (emit gather + transition lookups) only needs
  numpy gathers over host-resident inputs (~25 ms) and is computed on
  the host, keeping the device kernel purely the log-partition.
"""

import numpy as np
import ml_dtypes

B, S, T = 1024, 512, 48
NCORES = 8
BC = B // NCORES          # 128 batch rows per core
DELTA = 5.0               # deterministic per-step log shift
NT = 255                  # chain ticks (fwd 255 steps + bwd 256 steps)
CHUNK = 64                # steps per feat chunk
NPAIR = 4                 # chunk pairs (fwd c, bwd 7-c)
PAD = 128                 # qpad block width: fwd 48 | pad | bwd 48 | pad

BF16 = ml_dtypes.bfloat16

_NC = None


def _build_nc():
    import concourse.mybir as mybir
    import concourse.tile as tile
    from concourse import bacc

    f32 = mybir.dt.float32
    bf16 = mybir.dt.bfloat16
    Act = mybir.ActivationFunctionType
    Alu = mybir.AluOpType

    nc = bacc.Bacc()

    feats_d = nc.declare_dram_parameter("feats", [BC, S * T], f32, isOutput=False)
    wf_d = nc.declare_dram_parameter("wf", [T, 64], bf16, isOutput=False)
    wb_d = nc.declare_dram_parameter("wb", [T, 64], bf16, isOutput=False)
    initsc_d = nc.declare_dram_parameter("initsc", [128, 1], f32, isOutput=False)
    ones48_d = nc.declare_dram_parameter("ones48", [T, 1], bf16, isOutput=False)
    ones64_d = nc.declare_dram_parameter("ones64", [1, 64], bf16, isOutput=False)
    negdelta_d = nc.declare_dram_parameter("negdelta", [128, 1], f32, isOutput=False)
    zero1_d = nc.declare_dram_parameter("zero1", [1, 1], f32, isOutput=False)
    logz_d = nc.declare_dram_parameter("logz", [1, BC], f32, isOutput=True)

    with tile.TileContext(nc) as tc:
        with (
            tc.tile_pool(name="const", bufs=1) as cpool,
            tc.tile_pool(name="qtp", bufs=1) as qtp,
            tc.tile_pool(name="feat", bufs=2) as fpool,
            tc.tile_pool(name="qpad", bufs=2) as qpool,
            tc.tile_pool(name="state", bufs=3) as spool,
            tc.tile_pool(name="small", bufs=2) as mpool,
            tc.tile_pool(name="psum", bufs=2, space="PSUM") as psum,
            tc.tile_pool(name="psmall", bufs=2, space="PSUM") as pss,
        ):
            # ---- constants ----
            wf_sb = cpool.tile([T, 64], bf16, name="wf_sb")
            nc.sync.dma_start(wf_sb[:, :], wf_d[:, :])
            wb_sb = cpool.tile([112, 64], bf16, name="wb_sb")
            nc.sync.dma_start(wb_sb[64:112, :], wb_d[:, :])
            initsc_sb = cpool.tile([128, 1], f32, name="initsc_sb")
            nc.sync.dma_start(initsc_sb[:, :], initsc_d[:, :])
            ones48_sb = cpool.tile([T, 1], bf16, name="ones48_sb")
            nc.sync.dma_start(ones48_sb[:, :], ones48_d[:, :])
            ones48b_sb = cpool.tile([112, 1], bf16, name="ones48b_sb")
            nc.sync.dma_start(ones48b_sb[64:112, :], ones48_d[:, :])
            ones64_sb = cpool.tile([1, 64], bf16, name="ones64_sb")
            nc.sync.dma_start(ones64_sb[:, :], ones64_d[:, :])
            negdelta_sb = cpool.tile([128, 1], f32, name="negdelta_sb")
            nc.sync.dma_start(negdelta_sb[:, :], negdelta_d[:, :])
            zero1_sb = cpool.tile([1, 1], f32, name="zero1_sb")
            nc.sync.dma_start(zero1_sb[:, :], zero1_d[:, :])

            # ---- transposed Q storage: block j cols [128j:128j+128):
            #      rows 0-47 = Q^T(fwd step j), rows 64-111 = Q^T(bwd 511-j)
            qt = qtp.tile([128, 256 * PAD], bf16, name="qt")
            qt3 = qt.rearrange("p (b c) -> p b c", c=PAD)

            # ---- per block-range prep: DMA feats, exp, pad-memset, transpose
            # First ranges are small so the chain can start almost
            # immediately; later ranges amortize trigger overheads.
            def prep(b0, b1):
                n = b1 - b0
                ff = fpool.tile([BC, CHUNK * T], f32, tag="ff", name="ff")
                nc.sync.dma_start(
                    ff[:, 0:n * T], feats_d[:, b0 * T:b1 * T],
                )
                fb = fpool.tile([BC, CHUNK * T], f32, tag="fb", name="fb")
                nc.sync.dma_start(
                    fb[:, 0:n * T],
                    feats_d[:, (S - b1) * T:(S - b0) * T],
                )
                ff3 = ff.rearrange("p (s j) -> p s j", j=T)
                fb3 = fb.rearrange("p (s j) -> p s j", j=T)

                qp = qpool.tile([BC, CHUNK * PAD], bf16, tag="qp", name="qp")
                qp3 = qp.rearrange("p (s k) -> p s k", k=PAD)
                nc.gpsimd.memset(qp3[:, 0:n, T:64], 0.0)
                nc.gpsimd.memset(qp3[:, 0:n, 64 + T:PAD], 0.0)
                # bwd steps reversed so block j holds bwd step 511-j
                nc.scalar.activation(
                    qp3[:, 0:n, 64:64 + T], fb3[:, n - 1::-1, :], Act.Exp,
                    bias=negdelta_sb[:, :],
                )
                nc.scalar.activation(
                    qp3[:, 0:n, 0:T], ff3[:, 0:n, :], Act.Exp,
                    bias=negdelta_sb[:, :],
                )
                nc.scalar.dma_start(
                    qt3[:, b0:b1, :], qp[:, 0:n * PAD],
                    transpose=True,
                )

            for b0, b1 in ((0, 8), (8, 24), (24, 64),
                           (64, 128), (128, 192), (192, 256)):
                prep(b0, b1)

            # ---- init states: P_0 = Q_0 * expstart ; X_511 = Q_511 * expstop
            states = []
            for s2 in range(2):
                cols = slice(64 * s2, 64 * s2 + 64)
                st = spool.tile([128, 64], bf16, tag=f"st{s2}", name=f"st{s2}")
                nc.vector.tensor_scalar(
                    st[:, :], qt3[:, 0, cols], initsc_sb[:, :], None, Alu.mult
                )
                states.append(st)
            cacc = [None, None]

            # ---- the chain ----
            for k in range(NT):
                for s2 in range(2):
                    cols = slice(64 * s2, 64 * s2 + 64)
                    st = states[s2]
                    mm = psum.tile([128, 64], f32, tag=f"mm{s2}", name=f"mm{s2}")
                    nc.tensor.matmul(
                        mm[0:64, :], wf_sb[:, :], st[0:T, :],
                        start=True, stop=True, tile_position=(0, 0),
                    )
                    nc.tensor.matmul(
                        mm[64:128, :], wb_sb[64:112, :], st[64:64 + T, :],
                        start=True, stop=True, tile_position=(64, 64),
                    )
                    nst = spool.tile([128, 64], bf16, tag=f"st{s2}",
                                     name=f"nst{s2}")
                    nc.vector.tensor_tensor(
                        nst[:, :], mm[:, :], qt3[:, k + 1, cols], Alu.mult
                    )
                    states[s2] = nst

                    if k == 127:
                        # exact renorm: scale both halves by 1/colsum,
                        # bank the log-correction
                        st = states[s2]
                        msf = pss.tile([1, 64], f32, tag="ps_a", name="msf")
                        nc.tensor.matmul(
                            msf[:, :], ones48_sb[:, :], st[0:T, :],
                            start=True, stop=True, tile_position=(0, 0),
                        )
                        msb = pss.tile([1, 64], f32, tag="ps_b", name="msb")
                        nc.tensor.matmul(
                            msb[:, :], ones48b_sb[64:112, :], st[64:64 + T, :],
                            start=True, stop=True, tile_position=(64, 0),
                        )
                        lnf = mpool.tile([1, 64], f32, tag=f"lnf{s2}", name="lnf")
                        nc.scalar.activation(lnf[:, :], msf[:, :], Act.Ln, bias=zero1_sb[:, :])
                        lnb = mpool.tile([1, 64], f32, tag=f"lnb{s2}", name="lnb")
                        nc.scalar.activation(lnb[:, :], msb[:, :], Act.Ln, bias=zero1_sb[:, :])
                        cs = mpool.tile([1, 64], f32, tag=f"cs{s2}", name="cs")
                        nc.vector.tensor_add(cs[:, :], lnf[:, :], lnb[:, :])
                        cacc[s2] = cs
                        rf = mpool.tile([1, 64], f32, tag=f"rf{s2}", name="rf")
                        nc.vector.reciprocal(rf[:, :], msf[:, :])
                        rb = mpool.tile([1, 64], f32, tag=f"rb{s2}", name="rb")
                        nc.vector.reciprocal(rb[:, :], msb[:, :])
                        rfb = mpool.tile([1, 64], bf16, tag=f"rfb{s2}", name="rfb")
                        nc.vector.tensor_copy(rfb[:, :], rf[:, :])
                        rbb = mpool.tile([1, 64], bf16, tag=f"rbb{s2}", name="rbb")
                        nc.vector.tensor_copy(rbb[:, :], rb[:, :])
                        sc = psum.tile([128, 64], f32, tag=f"mm{s2}", name="sc")
                        nc.tensor.matmul(
                            sc[0:64, :], ones64_sb[:, :], rfb[:, :],
                            start=True, stop=True, tile_position=(0, 0),
                        )
                        nc.tensor.matmul(
                            sc[64:128, :], ones64_sb[:, :], rbb[:, :],
                            start=True, stop=True, tile_position=(0, 64),
                        )
                        nst2 = spool.tile([128, 64], bf16, tag=f"st{s2}",
                                          name=f"rst{s2}")
                        nc.vector.tensor_tensor(
                            nst2[:, :], sc[:, :], st[:, :], Alu.mult
                        )
                        states[s2] = nst2

            # ---- finalize: bv_255 = M^T X_256 ; Z = sum P_255 (.) bv_255
            logz_sb = cpool.tile([1, BC], f32, name="logz_sb")
            for s2 in range(2):
                cols = slice(64 * s2, 64 * s2 + 64)
                st = states[s2]
                bv = psum.tile([128, 64], f32, tag=f"mm{s2}", name="bv")
                nc.tensor.matmul(
                    bv[0:64, :], wb_sb[64:112, :], st[64:64 + T, :],
                    start=True, stop=True, tile_position=(64, 0),
                )
                zel = mpool.tile([T, 64], bf16, tag=f"zel{s2}", name="zel")
                nc.vector.tensor_tensor(
                    zel[:, :], bv[0:T, :], st[0:T, :], Alu.mult
                )
                zs = pss.tile([1, 64], f32, tag="ps_a", name="zs")
                nc.tensor.matmul(
                    zs[:, :], ones48_sb[:, :], zel[:, :],
                    start=True, stop=True, tile_position=(0, 0),
                )
                lnz = mpool.tile([1, 64], f32, tag=f"lnz{s2}", name="lnz")
                nc.scalar.activation(lnz[:, :], zs[:, :], Act.Ln, bias=zero1_sb[:, :])
                nc.vector.tensor_add(
                    logz_sb[:, cols], lnz[:, :], cacc[s2][:, :]
                )
            nc.sync.dma_start(logz_d[:, :], logz_sb[:, :])

    if not nc.is_finalized():
        nc.finalize()
    return nc


def _get_nc():
    global _NC
    if _NC is None:
        _NC = _build_nc()
    return _NC


def _in_maps(feats):
    Tm = _PARAMS["Tm"]
    st = _PARAMS["st"]
    sp = _PARAMS["sp"]
    M = np.exp(Tm)                          # M[next, cur]
    wf = np.zeros((T, 64), dtype=np.float32)
    wf[:, 0:T] = M.T                        # lhsT for out = M @ x
    wb = np.zeros((T, 64), dtype=np.float32)
    wb[:, 0:T] = M                          # lhsT for out = M.T @ x
    initsc = np.zeros((128, 1), dtype=np.float32)
    initsc[0:T, 0] = np.exp(st)
    initsc[64:64 + T, 0] = np.exp(sp)
    ones48 = np.ones((T, 1), dtype=BF16)
    ones64 = np.ones((1, 64), dtype=BF16)

    maps = []
    for i in range(NCORES):
        sl = slice(i * BC, (i + 1) * BC)
        maps.append(dict(
            feats=feats[sl].reshape(BC, S * T),
            wf=wf.astype(BF16),
            wb=wb.astype(BF16),
            initsc=initsc,
            ones48=ones48,
            ones64=ones64,
            negdelta=np.full((128, 1), -DELTA, dtype=np.float32),
            zero1=np.zeros((1, 1), dtype=np.float32),
        ))
    return maps


_PARAMS = {}


def kernel(feats, tags, mask, transitions, start_transitions, stop_transitions,
           _trace_tmpdir=None):
    from concourse.bass_utils import run_bass_kernel_spmd

    feats = np.ascontiguousarray(np.asarray(feats, dtype=np.float32))
    tags = np.asarray(tags).astype(np.int64)
    Tm = np.asarray(transitions, dtype=np.float32)
    st = np.asarray(start_transitions, dtype=np.float32)
    sp = np.asarray(stop_transitions, dtype=np.float32)
    _PARAMS.update(Tm=Tm, st=st, sp=sp)

    # host: full gold-path score (emit gather + transition lookups)
    emit = np.take_along_axis(
        feats.reshape(B, S * T),
        np.arange(S, dtype=np.int64)[None, :] * T + tags, axis=1,
    ).sum(axis=1, dtype=np.float64)
    gold = (
        emit
        + Tm[tags[:, 1:], tags[:, :-1]].sum(axis=1, dtype=np.float64)
        + st[tags[:, 0]].astype(np.float64)
        + sp[tags[:, -1]].astype(np.float64)
    )

    nc = _get_nc()
    maps = _in_maps(feats)
    if _trace_tmpdir is not None:
        return run_bass_kernel_spmd(nc, maps, list(range(NCORES)),
                                    trace=True, tmpdir=_trace_tmpdir)
    res = run_bass_kernel_spmd(nc, maps, list(range(NCORES))).results

    logz = np.concatenate([r["logz"][0].astype(np.float64) for r in res])
    logz = logz + S * DELTA
    loss = np.mean(logz - gold)
    return np.float32(loss)


# revision 18
# speedup vs baseline: 1.0613x; 1.0422x over previous
"""CRF negative log-likelihood on 8 Trainium2 NeuronCores.

Strategy (pure data parallel, batch sharded 1024 -> 8 x 128):

  The forward-algorithm recursion is run in the exp domain as
      P_{t} = Q_t (.) (M P_{t-1}),   Q = exp(feat - DELTA),  M = exp(T)
  i.e. one tiny PE matmul + one DVE multiply per step.  To double
  instruction-level parallelism and halve the serial chain length, the
  sequence is processed simultaneously from BOTH ends:
      forward  :  P_t  = Q_t (.) (M P_{t-1})          t = 1..255
      backward :  bv_t = M^T (Q_{t+1} (.) bv_{t+1})   t = 510..255
      Z_b = sum_j P_255[j,b] * bv_255[j,b]
  The fwd state lives on SBUF partitions 0..47 and the bwd state on
  64..111, so each tick needs ONE [128,64] DVE multiply per column
  stream (2 streams of 64 batch columns hide the PE<->DVE latency).
  Matmuls keep contraction k=48 on the partition ranges 0-47 / 64-111
  via PE tile_position quadrants, so the junk lanes between them are
  never consumed.

  Q is produced in natural layout by the scalar engine (exp with bias)
  into a 64-padded buffer and moved to [tag, batch] layout by ONE big
  multi-tile xbar DMA-transpose per 64-step chunk-pair (bwd steps are
  written in reversed order so transposed blocks come out tick-indexed).
  Exact renormalization once at mid-chain keeps bf16/fp32 in range; the
  log-corrections are added to logz at the end.

  The gold-path score (emit gather + transition lookups) only needs
  numpy gathers over host-resident inputs (~25 ms) and is computed on
  the host, keeping the device kernel purely the log-partition.
"""

import numpy as np
import ml_dtypes

B, S, T = 1024, 512, 48
NCORES = 8
BC = B // NCORES          # 128 batch rows per core
DELTA = 5.0               # deterministic per-step log shift
NT = 255                  # chain ticks (fwd 255 steps + bwd 256 steps)
CHUNK = 64                # steps per feat chunk
NPAIR = 4                 # chunk pairs (fwd c, bwd 7-c)
PAD = 128                 # qpad block width: fwd 48 | pad | bwd 48 | pad

BF16 = ml_dtypes.bfloat16

_NC = None


def _build_nc():
    import concourse.mybir as mybir
    import concourse.tile as tile
    from concourse import bacc

    f32 = mybir.dt.float32
    bf16 = mybir.dt.bfloat16
    Act = mybir.ActivationFunctionType
    Alu = mybir.AluOpType

    nc = bacc.Bacc()

    feats_d = nc.declare_dram_parameter("feats", [BC, S * T], f32, isOutput=False)
    wf_d = nc.declare_dram_parameter("wf", [T, 64], bf16, isOutput=False)
    wb_d = nc.declare_dram_parameter("wb", [T, 64], bf16, isOutput=False)
    initsc_d = nc.declare_dram_parameter("initsc", [128, 1], f32, isOutput=False)
    ones48_d = nc.declare_dram_parameter("ones48", [T, 1], bf16, isOutput=False)
    ones64_d = nc.declare_dram_parameter("ones64", [1, 64], bf16, isOutput=False)
    negdelta_d = nc.declare_dram_parameter("negdelta", [128, 1], f32, isOutput=False)
    zero1_d = nc.declare_dram_parameter("zero1", [1, 1], f32, isOutput=False)
    logz_d = nc.declare_dram_parameter("logz", [1, BC], f32, isOutput=True)

    with tile.TileContext(nc) as tc:
        with (
            tc.tile_pool(name="const", bufs=1) as cpool,
            tc.tile_pool(name="qtp", bufs=1) as qtp,
            tc.tile_pool(name="feat", bufs=2) as fpool,
            tc.tile_pool(name="qpad", bufs=2) as qpool,
            tc.tile_pool(name="state", bufs=3) as spool,
            tc.tile_pool(name="small", bufs=2) as mpool,
            tc.tile_pool(name="psum", bufs=2, space="PSUM") as psum,
            tc.tile_pool(name="psmall", bufs=2, space="PSUM") as pss,
        ):
            # ---- first feats pieces + chain-critical constants on sync ----
            RANGES = ((0, 8), (8, 24), (24, 64),
                      (64, 128), (128, 192), (192, 256))
            ffs, fbs = {}, {}
            for idx, (b0, b1) in enumerate(RANGES):
                n = b1 - b0
                # bulk ranges go via gpsimd (software DGE) to keep the
                # HWDGE free for the xbar transposes
                eng = nc.sync if idx < 2 else nc.gpsimd
                ff = fpool.tile([BC, CHUNK * T], f32, tag="ff", name="ff")
                eng.dma_start(ff[:, 0:n * T], feats_d[:, b0 * T:b1 * T])
                fb = fpool.tile([BC, CHUNK * T], f32, tag="fb", name="fb")
                eng.dma_start(fb[:, 0:n * T],
                              feats_d[:, (S - b1) * T:(S - b0) * T])
                ffs[b0], fbs[b0] = ff, fb
                if idx == 0:
                    wf_sb = cpool.tile([T, 64], bf16, name="wf_sb")
                    nc.sync.dma_start(wf_sb[:, :], wf_d[:, :])
                    wb_sb = cpool.tile([112, 64], bf16, name="wb_sb")
                    nc.sync.dma_start(wb_sb[64:112, :], wb_d[:, :])
                    initsc_sb = cpool.tile([128, 1], f32, name="initsc_sb")
                    nc.sync.dma_start(initsc_sb[:, :], initsc_d[:, :])
                    negdelta_sb = cpool.tile([128, 1], f32, name="negdelta_sb")
                    nc.sync.dma_start(negdelta_sb[:, :], negdelta_d[:, :])

            # remaining constants via the vector engine's queue
            ones48_sb = cpool.tile([T, 1], bf16, name="ones48_sb")
            nc.gpsimd.dma_start(ones48_sb[:, :], ones48_d[:, :])
            ones48b_sb = cpool.tile([112, 1], bf16, name="ones48b_sb")
            nc.gpsimd.dma_start(ones48b_sb[64:112, :], ones48_d[:, :])
            ones64_sb = cpool.tile([1, 64], bf16, name="ones64_sb")
            nc.gpsimd.dma_start(ones64_sb[:, :], ones64_d[:, :])
            zero1_sb = cpool.tile([1, 1], f32, name="zero1_sb")
            nc.gpsimd.dma_start(zero1_sb[:, :], zero1_d[:, :])

            # ---- transposed Q storage: block j cols [128j:128j+128):
            #      rows 0-47 = Q^T(fwd step j), rows 64-111 = Q^T(bwd 511-j)
            qt = qtp.tile([128, 256 * PAD], bf16, name="qt")
            qt3 = qt.rearrange("p (b c) -> p b c", c=PAD)

            # qp: two persistent buffers, pad columns zeroed exactly once
            qps = []
            for name in ("qpA", "qpB"):
                qp = qtp.tile([BC, CHUNK * PAD], bf16, name=name)
                qp3 = qp.rearrange("p (s k) -> p s k", k=PAD)
                nc.gpsimd.memset(qp3[:, :, T:64], 0.0)
                nc.gpsimd.memset(qp3[:, :, 64 + T:PAD], 0.0)
                qps.append((qp, qp3))

            # ---- per block-range prep: exp then one xbar transpose ----
            for idx, (b0, b1) in enumerate(RANGES):
                n = b1 - b0
                ff3 = ffs[b0].rearrange("p (s j) -> p s j", j=T)
                fb3 = fbs[b0].rearrange("p (s j) -> p s j", j=T)
                qp, qp3 = qps[idx % 2]
                # bwd steps reversed so block j holds bwd step 511-j
                nc.scalar.activation(
                    qp3[:, 0:n, 64:64 + T], fb3[:, n - 1::-1, :], Act.Exp,
                    bias=negdelta_sb[:, :],
                )
                nc.scalar.activation(
                    qp3[:, 0:n, 0:T], ff3[:, 0:n, :], Act.Exp,
                    bias=negdelta_sb[:, :],
                )
                nc.sync.dma_start(
                    qt3[:, b0:b1, :], qp[:, 0:n * PAD],
                    transpose=True,
                )

            # ---- init states: P_0 = Q_0 * expstart ; X_511 = Q_511 * expstop
            states = []
            for s2 in range(2):
                cols = slice(64 * s2, 64 * s2 + 64)
                st = spool.tile([128, 64], bf16, tag=f"st{s2}", name=f"st{s2}")
                nc.vector.tensor_scalar(
                    st[:, :], qt3[:, 0, cols], initsc_sb[:, :], None, Alu.mult
                )
                states.append(st)
            cacc = [None, None]

            # ---- the chain ----
            for k in range(NT):
                for s2 in range(2):
                    cols = slice(64 * s2, 64 * s2 + 64)
                    st = states[s2]
                    mm = psum.tile([128, 64], f32, tag=f"mm{s2}", name=f"mm{s2}")
                    nc.tensor.matmul(
                        mm[0:64, :], wf_sb[:, :], st[0:T, :],
                        start=True, stop=True, tile_position=(0, 0),
                    )
                    nc.tensor.matmul(
                        mm[64:128, :], wb_sb[64:112, :], st[64:64 + T, :],
                        start=True, stop=True, tile_position=(64, 64),
                    )
                    nst = spool.tile([128, 64], bf16, tag=f"st{s2}",
                                     name=f"nst{s2}")
                    nc.vector.tensor_tensor(
                        nst[:, :], mm[:, :], qt3[:, k + 1, cols], Alu.mult
                    )
                    states[s2] = nst

                    if k == 127:
                        # exact renorm: scale both halves by 1/colsum,
                        # bank the log-correction
                        st = states[s2]
                        msf = pss.tile([1, 64], f32, tag="ps_a", name="msf")
                        nc.tensor.matmul(
                            msf[:, :], ones48_sb[:, :], st[0:T, :],
                            start=True, stop=True, tile_position=(0, 0),
                        )
                        msb = pss.tile([1, 64], f32, tag="ps_b", name="msb")
                        nc.tensor.matmul(
                            msb[:, :], ones48b_sb[64:112, :], st[64:64 + T, :],
                            start=True, stop=True, tile_position=(64, 0),
                        )
                        lnf = mpool.tile([1, 64], f32, tag=f"lnf{s2}", name="lnf")
                        nc.scalar.activation(lnf[:, :], msf[:, :], Act.Ln, bias=zero1_sb[:, :])
                        lnb = mpool.tile([1, 64], f32, tag=f"lnb{s2}", name="lnb")
                        nc.scalar.activation(lnb[:, :], msb[:, :], Act.Ln, bias=zero1_sb[:, :])
                        cs = mpool.tile([1, 64], f32, tag=f"cs{s2}", name="cs")
                        nc.vector.tensor_add(cs[:, :], lnf[:, :], lnb[:, :])
                        cacc[s2] = cs
                        rf = mpool.tile([1, 64], f32, tag=f"rf{s2}", name="rf")
                        nc.vector.reciprocal(rf[:, :], msf[:, :])
                        rb = mpool.tile([1, 64], f32, tag=f"rb{s2}", name="rb")
                        nc.vector.reciprocal(rb[:, :], msb[:, :])
                        rfb = mpool.tile([1, 64], bf16, tag=f"rfb{s2}", name="rfb")
                        nc.vector.tensor_copy(rfb[:, :], rf[:, :])
                        rbb = mpool.tile([1, 64], bf16, tag=f"rbb{s2}", name="rbb")
                        nc.vector.tensor_copy(rbb[:, :], rb[:, :])
                        sc = psum.tile([128, 64], f32, tag=f"mm{s2}", name="sc")
                        nc.tensor.matmul(
                            sc[0:64, :], ones64_sb[:, :], rfb[:, :],
                            start=True, stop=True, tile_position=(0, 0),
                        )
                        nc.tensor.matmul(
                            sc[64:128, :], ones64_sb[:, :], rbb[:, :],
                            start=True, stop=True, tile_position=(0, 64),
                        )
                        nst2 = spool.tile([128, 64], bf16, tag=f"st{s2}",
                                          name=f"rst{s2}")
                        nc.vector.tensor_tensor(
                            nst2[:, :], sc[:, :], st[:, :], Alu.mult
                        )
                        states[s2] = nst2

            # ---- finalize: bv_255 = M^T X_256 ; Z = sum P_255 (.) bv_255
            logz_sb = cpool.tile([1, BC], f32, name="logz_sb")
            for s2 in range(2):
                cols = slice(64 * s2, 64 * s2 + 64)
                st = states[s2]
                bv = psum.tile([128, 64], f32, tag=f"mm{s2}", name="bv")
                nc.tensor.matmul(
                    bv[0:64, :], wb_sb[64:112, :], st[64:64 + T, :],
                    start=True, stop=True, tile_position=(64, 0),
                )
                zel = mpool.tile([T, 64], bf16, tag=f"zel{s2}", name="zel")
                nc.vector.tensor_tensor(
                    zel[:, :], bv[0:T, :], st[0:T, :], Alu.mult
                )
                zs = pss.tile([1, 64], f32, tag="ps_a", name="zs")
                nc.tensor.matmul(
                    zs[:, :], ones48_sb[:, :], zel[:, :],
                    start=True, stop=True, tile_position=(0, 0),
                )
                lnz = mpool.tile([1, 64], f32, tag=f"lnz{s2}", name="lnz")
                nc.scalar.activation(lnz[:, :], zs[:, :], Act.Ln, bias=zero1_sb[:, :])
                nc.vector.tensor_add(
                    logz_sb[:, cols], lnz[:, :], cacc[s2][:, :]
                )
            nc.sync.dma_start(logz_d[:, :], logz_sb[:, :])

    if not nc.is_finalized():
        nc.finalize()
    return nc


def _get_nc():
    global _NC
    if _NC is None:
        _NC = _build_nc()
    return _NC


def _in_maps(feats):
    Tm = _PARAMS["Tm"]
    st = _PARAMS["st"]
    sp = _PARAMS["sp"]
    M = np.exp(Tm)                          # M[next, cur]
    wf = np.zeros((T, 64), dtype=np.float32)
    wf[:, 0:T] = M.T                        # lhsT for out = M @ x
    wb = np.zeros((T, 64), dtype=np.float32)
    wb[:, 0:T] = M                          # lhsT for out = M.T @ x
    initsc = np.zeros((128, 1), dtype=np.float32)
    initsc[0:T, 0] = np.exp(st)
    initsc[64:64 + T, 0] = np.exp(sp)
    ones48 = np.ones((T, 1), dtype=BF16)
    ones64 = np.ones((1, 64), dtype=BF16)

    maps = []
    for i in range(NCORES):
        sl = slice(i * BC, (i + 1) * BC)
        maps.append(dict(
            feats=feats[sl].reshape(BC, S * T),
            wf=wf.astype(BF16),
            wb=wb.astype(BF16),
            initsc=initsc,
            ones48=ones48,
            ones64=ones64,
            negdelta=np.full((128, 1), -DELTA, dtype=np.float32),
            zero1=np.zeros((1, 1), dtype=np.float32),
        ))
    return maps


_PARAMS = {}


def kernel(feats, tags, mask, transitions, start_transitions, stop_transitions,
           _trace_tmpdir=None):
    from concourse.bass_utils import run_bass_kernel_spmd

    feats = np.ascontiguousarray(np.asarray(feats, dtype=np.float32))
    tags = np.asarray(tags).astype(np.int64)
    Tm = np.asarray(transitions, dtype=np.float32)
    st = np.asarray(start_transitions, dtype=np.float32)
    sp = np.asarray(stop_transitions, dtype=np.float32)
    _PARAMS.update(Tm=Tm, st=st, sp=sp)

    # host: full gold-path score (emit gather + transition lookups)
    emit = np.take_along_axis(
        feats.reshape(B, S * T),
        np.arange(S, dtype=np.int64)[None, :] * T + tags, axis=1,
    ).sum(axis=1, dtype=np.float64)
    gold = (
        emit
        + Tm[tags[:, 1:], tags[:, :-1]].sum(axis=1, dtype=np.float64)
        + st[tags[:, 0]].astype(np.float64)
        + sp[tags[:, -1]].astype(np.float64)
    )

    nc = _get_nc()
    maps = _in_maps(feats)
    if _trace_tmpdir is not None:
        return run_bass_kernel_spmd(nc, maps, list(range(NCORES)),
                                    trace=True, tmpdir=_trace_tmpdir)
    res = run_bass_kernel_spmd(nc, maps, list(range(NCORES))).results

    logz = np.concatenate([r["logz"][0].astype(np.float64) for r in res])
    logz = logz + S * DELTA
    loss = np.mean(logz - gold)
    return np.float32(loss)
